# revision 3
# baseline (speedup 1.0000x reference)
"""Trainium2 Bass kernel for AdaAttentionalPropagation (masked multi-head
cross-attention + merge conv + MLP with InstanceNorm/ReLU).

Full inputs in, full output out. Internally: data-parallel over batch B=8
across 8 NeuronCores (one batch element per core, no collectives).

Math notes (host-side folds, all exact):
  - head channels are re-permuted to blocked layout (h*64+d) by permuting
    Wq/Wk/Wv rows and Wm columns
  - 1/sqrt(dh) is folded into Wq and bq
  - bv folds into an effective merge bias bmE = Wm@bv + bm (softmax rows sum
    to 1, so v's bias contributes Wm@bv to the message)
  - b1 is dropped: a per-channel constant cancels in InstanceNorm(affine=False)
  - softmax is computed without max-subtraction (scores are O(1) here)
  - softmax denominator comes free from a ones-column appended to v^T in the
    attention matmul (row 64 of the PSUM accumulator)
  - mask is carried in fp8e4m3 (validated: ~3e-4 rel err end to end); halves
    the dominant input DMA so the pipeline start isn't mask-gated

Schedule notes:
  - window = 128 iterations of {scores MM pair (row-tiled, concurrent),
    DVE mask-mult from PSUM (1142ns, the pipeline governor), Scalar exp
    on [128,2048] double-tiles, trailing attention MMs}
  - q/k bias-adds run on DVE in the prologue; vt conversion on Scalar
  - per-pass drains are immediate (frees the 2 accumulator banks); the
    reciprocal/normalize finish is deferred ~4 iterations into the next pass
  - merge conv h0 + MLP1 h0 matmuls are interleaved into passes 4-7 PE slack
  - tail: merge h1, MLP1 h1, InstanceNorm stats (DVE, from SBUF), affine via
    ln/exp (same ACT table set as Exp -> no table reload), ReLU, MLP2, DMA out
"""

import sys

for _p in ("/opt/trn_rl_repo", "/root/.axon_site/_ro/trn_rl_repo"):
    if _p not in sys.path:
        sys.path.append(_p)

import numpy as np
import ml_dtypes
from contextlib import ExitStack

import concourse.bass as bass
import concourse.tile as tile
from concourse import bacc, mybir
from concourse.bass_utils import run_bass_kernel_spmd

B, D, N, NKV, H = 8, 256, 2048, 2048, 4
DH = D // H
EPS = 1e-5
NCORES = 8

BF = mybir.dt.bfloat16
F32 = mybir.dt.float32
F8 = mybir.dt.float8e4
AF = mybir.ActivationFunctionType
ALU = mybir.AluOpType
NPBF = ml_dtypes.bfloat16
NPF8 = ml_dtypes.float8_e4m3

_CACHE = {}


def _build():
    nc = bacc.Bacc("TRN2", target_bir_lowering=False, debug=False,
                   num_devices=NCORES)

    d_x = nc.dram_tensor("x", [128, 2, N], BF, kind="ExternalInput")
    d_src = nc.dram_tensor("src", [128, 2, N], BF, kind="ExternalInput")
    d_mask = nc.dram_tensor("maskT", [128, 16, N], F8, kind="ExternalInput")
    d_wq = nc.dram_tensor("wqT", [128, 2, 256], BF, kind="ExternalInput")
    d_wk = nc.dram_tensor("wkT", [128, 2, 256], BF, kind="ExternalInput")
    d_wv = nc.dram_tensor("wvT", [128, 2, 256], BF, kind="ExternalInput")
    d_wm = nc.dram_tensor("wmT", [128, 2, 256], BF, kind="ExternalInput")
    d_w1 = nc.dram_tensor("w1T", [128, 4, 512], BF, kind="ExternalInput")
    d_w2 = nc.dram_tensor("w2T", [128, 4, 256], BF, kind="ExternalInput")
    d_bq = nc.dram_tensor("bq", [128, 2], F32, kind="ExternalInput")
    d_bk = nc.dram_tensor("bk", [128, 2], F32, kind="ExternalInput")
    d_bm = nc.dram_tensor("bmE", [128, 2], F32, kind="ExternalInput")
    d_out = nc.dram_tensor("out", [128, 2, N], F32, kind="ExternalOutput")
    d_rscr = nc.dram_tensor("rscratch", [16, 512], F32)
    d_sums = nc.dram_tensor("sscratch", [16, 512], F32)

    with tile.TileContext(nc) as tc, ExitStack() as ctx:
        consts = ctx.enter_context(tc.tile_pool(name="consts", bufs=1))
        probp = ctx.enter_context(tc.tile_pool(name="probp", bufs=4))
        recp = ctx.enter_context(tc.tile_pool(name="recp", bufs=2))
        rbb = ctx.enter_context(tc.tile_pool(name="rbb", bufs=2))
        stgp = ctx.enter_context(tc.tile_pool(name="stgp", bufs=4))
        statp = ctx.enter_context(tc.tile_pool(name="statp", bufs=8))
        outp = ctx.enter_context(tc.tile_pool(name="outp", bufs=2))

        wq_sb = consts.tile([128, 2, 256], BF)
        wk_sb = consts.tile([128, 2, 256], BF)
        wv_sb = consts.tile([128, 2, 256], BF)
        wm_sb = consts.tile([128, 2, 256], BF)
        w1_sb = consts.tile([128, 4, 512], BF)
        w2_sb = consts.tile([128, 4, 256], BF)
        bq_sb = consts.tile([128, 2], F32)
        bk_sb = consts.tile([128, 2], F32)
        bm_sb = consts.tile([128, 2], F32)
        x_sb = consts.tile([128, 2, N], BF)
        src_sb = consts.tile([128, 2, N], BF)
        mask_sb = consts.tile([128, 16, N], F8)
        q_sb = consts.tile([128, 2, N], BF)
        k_sb = consts.tile([128, 2, N], BF)
        vt_sb = consts.tile([128, 16, H, DH + 1], BF)
        attn_sb = consts.tile([128, 2, N], BF)
        msg_sb = consts.tile([128, 2, N], BF)
        y1_sb = consts.tile([128, 4, N], BF)
        y1n_sb = consts.tile([128, 4, N], BF)
        eps_sb = consts.tile([128, 1], F32)

        # ---- input DMA: weights/x/src first (prologue dependencies), then
        # the fp8 mask chunks (needed from the first window pass on), then
        # the late-used MLP weights
        nc.sync.dma_start(out=wq_sb[:], in_=d_wq[:])
        nc.sync.dma_start(out=bq_sb[:], in_=d_bq[:])
        for q4 in range(4):
            nc.sync.dma_start(out=x_sb[:, 0, q4 * 512:(q4 + 1) * 512],
                              in_=d_x[:, 0, q4 * 512:(q4 + 1) * 512])
        nc.sync.dma_start(out=wk_sb[:], in_=d_wk[:])
        nc.sync.dma_start(out=bk_sb[:], in_=d_bk[:])
        for q4 in range(4):
            nc.sync.dma_start(out=src_sb[:, 0, q4 * 512:(q4 + 1) * 512],
                              in_=d_src[:, 0, q4 * 512:(q4 + 1) * 512])
        nc.sync.dma_start(out=wv_sb[:], in_=d_wv[:])
        for q4 in range(4):
            nc.sync.dma_start(out=x_sb[:, 1, q4 * 512:(q4 + 1) * 512],
                              in_=d_x[:, 1, q4 * 512:(q4 + 1) * 512])
        for q4 in range(4):
            nc.sync.dma_start(out=src_sb[:, 1, q4 * 512:(q4 + 1) * 512],
                              in_=d_src[:, 1, q4 * 512:(q4 + 1) * 512])
        for mc in range(16):
            nc.sync.dma_start(out=mask_sb[:, mc, :], in_=d_mask[:, mc, :])
        for w_sb, d_w in ((wm_sb, d_wm), (bm_sb, d_bm), (w1_sb, d_w1),
                          (w2_sb, d_w2)):
            nc.sync.dma_start(out=w_sb[:], in_=d_w[:])

        nc.vector.memset(eps_sb[:], EPS)
        nc.vector.memset(vt_sb[:, :, :, DH:DH + 1], 1.0)

        def bias_bcast(b_sb, oc, ncols):
            bb = b_sb[:, oc:oc + 1]
            return bass.AP(tensor=bb.tensor, offset=bb.offset,
                           ap=[list(bb.ap[0]), [0, ncols]])

        with tc.tile_pool(name="psA", bufs=2, space="PSUM") as psA, \
             tc.tile_pool(name="psB", bufs=2, space="PSUM") as psB, \
             tc.tile_pool(name="psC", bufs=1, space="PSUM") as psC:
            # ---- projections (prologue) ----
            # bias-adds run on DVE (idle until the window starts); vt
            # conversion runs on Scalar (its queue is free until the exps)
            def proj_qk(w_sb, b_sb, rhs_sb, dst, oc):
                for q4 in range(4):
                    pp = psB.tile([128, 512], F32, tag="psB")
                    n0 = q4 * 512
                    for kc in range(2):
                        nc.tensor.matmul(
                            pp[:],
                            lhsT=w_sb[:, kc, oc * 128:(oc + 1) * 128],
                            rhs=rhs_sb[:, kc, n0:n0 + 512],
                            start=(kc == 0), stop=(kc == 1))
                    nc.vector.tensor_tensor(
                        dst[:, oc, n0:n0 + 512], pp[:],
                        bias_bcast(b_sb, oc, 512), op=ALU.add)

            def make_vt(mc):
                pv = psB.tile([128, 256], F32, tag="psB")
                for kc in range(2):
                    nc.tensor.matmul(
                        pv[:],
                        lhsT=src_sb[:, kc, mc * 128:(mc + 1) * 128],
                        rhs=wv_sb[:, kc, :],
                        start=(kc == 0), stop=(kc == 1))
                nc.scalar.activation(
                    vt_sb[:, mc, :, 0:DH],
                    pv[:].rearrange("p (h d) -> p h d", h=H), AF.Copy)

            proj_qk(wq_sb, bq_sb, x_sb, q_sb, 0)
            proj_qk(wk_sb, bk_sb, src_sb, k_sb, 0)
            for mc in range(16):
                make_vt(mc)
            proj_qk(wq_sb, bq_sb, x_sb, q_sb, 1)
            proj_qk(wk_sb, bk_sb, src_sb, k_sb, 1)

            # ---- merge / MLP1 half-0 fillers (emitted inside passes 4-7)
            y1_done = {}

            def merge_grp(oc, half):
                mp = psC.tile([128, 1024], F32, tag="psC")
                for nq in range(2):
                    n0 = half * 1024 + nq * 512
                    for kc in range(2):
                        nc.tensor.matmul(
                            mp[:, nq * 512:(nq + 1) * 512],
                            lhsT=wm_sb[:, kc, oc * 128:(oc + 1) * 128],
                            rhs=attn_sb[:, kc, n0:n0 + 512],
                            start=(kc == 0), stop=(kc == 1))
                nc.scalar.activation(
                    msg_sb[:, oc, half * 1024:(half + 1) * 1024],
                    mp[:], AF.Identity, bias=bm_sb[:, oc:oc + 1])

            def y1_grp(oc, half):
                yp = psC.tile([128, 1024], F32, tag="psC")
                for nq in range(2):
                    n0 = half * 1024 + nq * 512
                    for kc in range(4):
                        rhs_sb2 = x_sb if kc < 2 else msg_sb
                        nc.tensor.matmul(
                            yp[:, nq * 512:(nq + 1) * 512],
                            lhsT=w1_sb[:, kc, oc * 128:(oc + 1) * 128],
                            rhs=rhs_sb2[:, kc % 2, n0:n0 + 512],
                            start=(kc == 0), stop=(kc == 3))
                nc.scalar.activation(
                    y1_sb[:, oc, half * 1024:(half + 1) * 1024],
                    yp[:], AF.Copy)
                y1_done[(oc, half)] = True

            # filler slots: (pass_index, mc) -> emit callable
            fillers = {
                (5, 5): lambda: merge_grp(0, 0),
                (5, 11): lambda: merge_grp(1, 0),
                (6, 5): lambda: y1_grp(0, 0),
                (6, 11): lambda: y1_grp(1, 0),
                (7, 5): lambda: y1_grp(2, 0),
                (7, 11): lambda: y1_grp(3, 0),
            }

            # ---- attention ----
            passes = [(0, 0), (0, 1), (1, 0), (1, 1),
                      (0, 2), (0, 3), (1, 2), (1, 3)]
            pending = []            # (pt2, ap_e, ap_o, hc, mc_even)
            finish_q = []           # deferred reciprocal/normalize closures

            def flush_attn():
                pt2, ap_e, ap_o, hc, mce = pending.pop(0)
                for j in range(2):
                    mc = mce + j
                    nc.tensor.matmul(
                        ap_e[:], lhsT=vt_sb[:, mc, 2 * hc, :],
                        rhs=pt2[:, j * 1024:j * 1024 + 512],
                        start=(mc == 0), stop=(mc == 15))
                    nc.tensor.matmul(
                        ap_o[:], lhsT=vt_sb[:, mc, 2 * hc + 1, :],
                        rhs=pt2[:, j * 1024 + 512:(j + 1) * 1024],
                        start=(mc == 0), stop=(mc == 15))

            def drain_pass(ap_e, ap_o, hc, nq4, pi):
                # immediate: PSUM -> SBUF staging + exp-sum row to DRAM +
                # the [1,512]->[128,4] reshape DMA back in. Frees ap banks.
                n0 = nq4 * 512
                items = []
                for side, ap_t in ((0, ap_e), (1, ap_o)):
                    ri = pi * 2 + side
                    stg = stgp.tile([65, 512], F32, tag="stg")
                    nc.scalar.activation(stg[:], ap_t[:], AF.Copy)
                    nc.sync.dma_start(out=d_sums[ri:ri + 1, :],
                                      in_=stg[64:65, :])
                    rtmp = recp.tile([128, 4], F32, tag="rtmp")
                    nc.sync.dma_start(
                        out=rtmp[:],
                        in_=d_sums[ri:ri + 1, :].rearrange(
                            "a (p c) -> (a p) c", p=128))
                    items.append((side, stg, rtmp, ri))

                def finish():
                    for side, stg, rtmp, ri in items:
                        hp = side * 64
                        rcp = recp.tile([128, 4], F32, tag="rcp")
                        nc.vector.reciprocal(rcp[:], rtmp[:])
                        nc.sync.dma_start(
                            out=d_rscr[ri:ri + 1, :].rearrange(
                                "a (p c) -> (a p) c", p=128),
                            in_=rcp[:])
                        rsc = d_rscr.ap()
                        bcast = bass.AP(tensor=rsc.tensor, offset=ri * 512,
                                        ap=[[0, 64], [1, 512]])
                        rb = rbb.tile([64, 512], F32, tag="rb")
                        nc.sync.dma_start(out=rb[:], in_=bcast)
                        nc.gpsimd.tensor_tensor(
                            attn_sb[hp:hp + 64, hc, n0:n0 + 512],
                            stg[0:64, :], rb[:], op=ALU.mult)
                return finish

            for pi, (hc, nq4) in enumerate(passes):
                n0 = nq4 * 512
                ap_e = psB.tile([65, 512], F32, tag="psB")
                ap_o = psB.tile([65, 512], F32, tag="psB")
                pt = None
                for mc in range(16):
                    sp = psA.tile([128, 1024], F32, tag="psA")
                    nc.tensor.matmul(
                        sp[:, 0:512],
                        lhsT=k_sb[0:64, hc, mc * 128:(mc + 1) * 128],
                        rhs=q_sb[0:64, hc, n0:n0 + 512],
                        tile_position=(0, 0))
                    nc.tensor.matmul(
                        sp[:, 512:1024],
                        lhsT=k_sb[64:128, hc, mc * 128:(mc + 1) * 128],
                        rhs=q_sb[64:128, hc, n0:n0 + 512],
                        tile_position=(64, 0))
                    while len(pending) >= 2:
                        flush_attn()
                    if mc == 4 and finish_q:
                        finish_q.pop(0)()
                    if (pi, mc) in fillers:
                        fillers[(pi, mc)]()
                    if mc % 2 == 0:
                        pt = probp.tile([128, 2048], BF, tag="pt")
                    off = (mc % 2) * 1024
                    mrow = mask_sb[:, mc, n0:n0 + 512]
                    mb = bass.AP(tensor=mrow.tensor, offset=mrow.offset,
                                 ap=[list(mrow.ap[0]), [0, 2], [1, 512]])
                    nc.vector.tensor_tensor(
                        pt[:, off:off + 1024].rearrange(
                            "p (t n) -> p t n", t=2),
                        sp[:].rearrange("p (t n) -> p t n", t=2),
                        mb, op=ALU.mult)
                    if mc % 2 == 1:
                        pt2 = probp.tile([128, 2048], BF, tag="pt2")
                        nc.scalar.activation(pt2[:], pt[:], AF.Exp)
                        pending.append((pt2, ap_e, ap_o, hc, mc - 1))
                while pending:
                    flush_attn()
                finish_q.append(drain_pass(ap_e, ap_o, hc, nq4, pi))
            while finish_q:
                finish_q.pop(0)()

            # ---- tail: merge h1, MLP1 h1, InstanceNorm, ReLU, MLP2 ----
            merge_grp(0, 1)
            merge_grp(1, 1)

            def y1_norm(oc):
                st = statp.tile([128, 4, 6], F32, tag="st")
                for half in range(2):
                    for nq in range(2):
                        n0 = half * 1024 + nq * 512
                        nc.vector.bn_stats(st[:, half * 2 + nq, :],
                                           y1_sb[:, oc, n0:n0 + 512])
                mv = statp.tile([128, 2], F32, tag="mv")
                nc.vector.bn_aggr(mv[:], st[:])
                # rs = 1/sqrt(var+eps) = exp(-0.5*ln(var+eps)); stays inside
                # the exp/ln ACT table set (no table reload after the exps)
                lg = statp.tile([128, 1], F32, tag="lg")
                nc.scalar.activation(lg[:], mv[:, 1:2], AF.Ln,
                                     bias=eps_sb[:])
                rs = statp.tile([128, 1], F32, tag="rs")
                nc.scalar.activation(rs[:], lg[:], AF.Exp, scale=-0.5)
                nb = statp.tile([128, 1], F32, tag="nb")
                nc.vector.scalar_tensor_tensor(nb[:], mv[:, 0:1], -1.0, rs[:],
                                               op0=ALU.mult, op1=ALU.mult)
                for half in range(2):
                    nc.scalar.activation(
                        y1n_sb[:, oc, half * 1024:(half + 1) * 1024],
                        y1_sb[:, oc, half * 1024:(half + 1) * 1024],
                        AF.Relu, bias=nb[:], scale=rs[:])

            y1_grp(0, 1)
            y1_norm(0)
            y1_grp(1, 1)
            y1_norm(1)
            y1_grp(2, 1)
            y1_norm(2)
            y1_grp(3, 1)
            y1_norm(3)

            for oc in range(2):
                for half in range(2):
                    op_t = psA.tile([128, 1024], F32, tag="psA")
                    for nq in range(2):
                        n0 = half * 1024 + nq * 512
                        for kc in range(4):
                            nc.tensor.matmul(
                                op_t[:, nq * 512:(nq + 1) * 512],
                                lhsT=w2_sb[:, kc, oc * 128:(oc + 1) * 128],
                                rhs=y1n_sb[:, kc, n0:n0 + 512],
                                start=(kc == 0), stop=(kc == 3))
                    o_sb = outp.tile([128, 1024], F32, tag="outsb")
                    nc.scalar.activation(o_sb[:], op_t[:], AF.Copy)
                    n0 = half * 1024
                    nc.sync.dma_start(out=d_out[:, oc, n0:n0 + 512],
                                      in_=o_sb[:, 0:512])
                    nc.sync.dma_start(out=d_out[:, oc, n0 + 512:n0 + 1024],
                                      in_=o_sb[:, 512:1024])

    nc.compile()
    return nc


def _chunk(a, p=128):
    # [C, ...] -> [128, C//128, ...] with partition-major layout
    c = a.shape[0]
    return np.ascontiguousarray(
        a.reshape(c // p, p, *a.shape[1:]).swapaxes(0, 1))


def _prep_inputs(x, source, mask, Wq, bq, Wk, bk, Wv, bv, Wm, bm, W1, b1,
                 W2, b2):
    # blocked-head channel permutation: new[h*64+d] = old[d*4+h]
    perm = (np.arange(DH)[None, :] * H + np.arange(H)[:, None]).reshape(-1)
    scale = 1.0 / np.sqrt(np.float32(DH))

    wq_t = _chunk((Wq[perm, :] * scale).T.astype(NPBF))
    wk_t = _chunk(Wk[perm, :].T.astype(NPBF))
    wv_t = _chunk(Wv[perm, :].T.astype(NPBF))
    wm_t = _chunk(Wm[:, perm].T.astype(NPBF))
    w1_t = _chunk(W1.T.astype(NPBF))
    w2_t = _chunk(W2.T.astype(NPBF))
    bq_t = _chunk((bq[perm] * scale).astype(np.float32))
    bk_t = _chunk(bk[perm].astype(np.float32))
    bm_t = _chunk((Wm @ bv + bm).astype(np.float32))

    shared = {"wqT": wq_t, "wkT": wk_t, "wvT": wv_t, "wmT": wm_t,
              "w1T": w1_t, "w2T": w2_t, "bq": bq_t, "bk": bk_t, "bmE": bm_t}

    in_maps = []
    for b in range(B):
        m = dict(shared)
        m["x"] = _chunk(np.asarray(x[b]).astype(NPBF))
        m["src"] = _chunk(np.asarray(source[b]).astype(NPBF))
        m["maskT"] = _chunk(np.ascontiguousarray(
            np.asarray(mask[b]).T).astype(NPF8))
        in_maps.append(m)
    return in_maps


def run(inputs, trace=False):
    if "nc" not in _CACHE:
        _CACHE["nc"] = _build()
    nc = _CACHE["nc"]
    in_maps = _prep_inputs(**inputs)
    res = run_bass_kernel_spmd(nc, in_maps, list(range(NCORES)), trace=trace)
    out = np.empty((B, D, N), np.float32)
    for b in range(B):
        o = res.results[b]["out"]  # [128, 2, N]
        out[b] = o.swapaxes(0, 1).reshape(D, N)
    return out, res


def kernel(**inputs):
    out, _ = run(inputs, trace=False)
    return out


# revision 4
# speedup vs baseline: 1.1110x; 1.1110x over previous
"""Trainium2 Bass kernel for AdaAttentionalPropagation (masked multi-head
cross-attention + merge conv + MLP with InstanceNorm/ReLU).

Full inputs in, full output out. Internally: data-parallel over batch B=8
across 8 NeuronCores (one batch element per core, no collectives).

Math notes (host-side folds, all exact):
  - head channels are re-permuted to blocked layout (h*64+d) by permuting
    Wq/Wk/Wv rows and Wm columns
  - 1/sqrt(dh) is folded into Wq and bq
  - bv folds into an effective merge bias bmE = Wm@bv + bm (softmax rows sum
    to 1, so v's bias contributes Wm@bv to the message)
  - b1 is dropped: a per-channel constant cancels in InstanceNorm(affine=False)
  - softmax is computed without max-subtraction (scores are O(1) here)
  - softmax denominator comes free from a ones-column appended to v^T in the
    attention matmul (row 64 of the PSUM accumulator)
  - mask and source are carried in fp8e4m3 (validated: ~4e-4 rel err end to
    end); shrinks the dominant input DMA so the pipeline start isn't gated

Schedule notes:
  - window = 128 iterations of {scores MM pair (row-tiled, concurrent),
    DVE mask-mult from PSUM (~1142ns, the pipeline governor), Scalar exp on
    [128,2048] double-tiles, trailing attention MMs}
  - prologue computes q/k chunk 0 (DVE bias-adds) + vT only; q/k chunk 1 is
    projected inside passes 0-1 from PE slack (Scalar bias)
  - per-pass accumulator drains are immediate (3-slot psB rotation never
    blocks the next pass); reciprocal/normalize finish is deferred ~4
    iterations into the next pass
  - merge conv h0 + MLP1 h0 matmuls run inside passes 5-7 from PE slack
  - tail: merge h1, MLP1 h1, InstanceNorm (stats on DVE from SBUF, Sqrt
    table-load hidden behind the pass-7 drain via a dummy activation),
    ReLU, MLP2 (DVE output copies), DMA out
"""

import sys

for _p in ("/opt/trn_rl_repo", "/root/.axon_site/_ro/trn_rl_repo"):
    if _p not in sys.path:
        sys.path.append(_p)

import numpy as np
import ml_dtypes
from contextlib import ExitStack

import concourse.bass as bass
import concourse.tile as tile
from concourse import bacc, mybir
from concourse.bass_utils import run_bass_kernel_spmd

B, D, N, NKV, H = 8, 256, 2048, 2048, 4
DH = D // H
EPS = 1e-5
NCORES = 8

BF = mybir.dt.bfloat16
F32 = mybir.dt.float32
F8 = mybir.dt.float8e4
AF = mybir.ActivationFunctionType
ALU = mybir.AluOpType
NPBF = ml_dtypes.bfloat16
NPF8 = ml_dtypes.float8_e4m3

_CACHE = {}


def _build():
    nc = bacc.Bacc("TRN2", target_bir_lowering=False, debug=False,
                   num_devices=NCORES)

    d_x = nc.dram_tensor("x", [128, 2, N], BF, kind="ExternalInput")
    d_src = nc.dram_tensor("src", [128, 2, N], F8, kind="ExternalInput")
    d_mask = nc.dram_tensor("maskT", [128, 16, N], F8, kind="ExternalInput")
    d_wq = nc.dram_tensor("wqT", [128, 2, 256], BF, kind="ExternalInput")
    d_wk = nc.dram_tensor("wkT", [128, 2, 256], BF, kind="ExternalInput")
    d_wv = nc.dram_tensor("wvT", [128, 2, 256], BF, kind="ExternalInput")
    d_wm = nc.dram_tensor("wmT", [128, 2, 256], BF, kind="ExternalInput")
    d_w1 = nc.dram_tensor("w1T", [128, 4, 512], BF, kind="ExternalInput")
    d_w2 = nc.dram_tensor("w2T", [128, 4, 256], BF, kind="ExternalInput")
    d_bq = nc.dram_tensor("bq", [128, 2], F32, kind="ExternalInput")
    d_bk = nc.dram_tensor("bk", [128, 2], F32, kind="ExternalInput")
    d_bm = nc.dram_tensor("bmE", [128, 2], F32, kind="ExternalInput")
    d_out = nc.dram_tensor("out", [128, 2, N], F32, kind="ExternalOutput")
    d_rscr = nc.dram_tensor("rscratch", [16, 512], F32)
    d_sums = nc.dram_tensor("sscratch", [16, 512], F32)

    with tile.TileContext(nc) as tc, ExitStack() as ctx:
        consts = ctx.enter_context(tc.tile_pool(name="consts", bufs=1))
        probp = ctx.enter_context(tc.tile_pool(name="probp", bufs=4))
        recp = ctx.enter_context(tc.tile_pool(name="recp", bufs=2))
        rbb = ctx.enter_context(tc.tile_pool(name="rbb", bufs=2))
        stgp = ctx.enter_context(tc.tile_pool(name="stgp", bufs=4))
        statp = ctx.enter_context(tc.tile_pool(name="statp", bufs=8))
        outp = ctx.enter_context(tc.tile_pool(name="outp", bufs=2))

        wq_sb = consts.tile([128, 2, 256], BF)
        wk_sb = consts.tile([128, 2, 256], BF)
        wv_sb = consts.tile([128, 2, 256], BF)
        wm_sb = consts.tile([128, 2, 256], BF)
        w1_sb = consts.tile([128, 4, 512], BF)
        w2_sb = consts.tile([128, 4, 256], BF)
        bq_sb = consts.tile([128, 2], F32)
        bk_sb = consts.tile([128, 2], F32)
        bm_sb = consts.tile([128, 2], F32)
        x_sb = consts.tile([128, 2, N], BF)
        src_sb = consts.tile([128, 2, N], F8)
        mask_sb = consts.tile([128, 16, N], F8)
        q_sb = consts.tile([128, 2, N], BF)
        k_sb = consts.tile([128, 2, N], BF)
        vt_sb = consts.tile([128, 16, H, DH + 1], BF)
        attn_sb = consts.tile([128, 2, N], BF)
        msg_sb = consts.tile([128, 2, N], BF)
        y1_sb = consts.tile([128, 4, N], BF)
        y1n_sb = consts.tile([128, 4, N], BF)
        eps_sb = consts.tile([128, 1], F32)
        scr_sb = consts.tile([128, 1], F32)

        # ---- input DMA. Order matters: prologue deps (wq/x-kc0/wk/src-kc0/
        # wv) first, then mask chunks interleaved with the kc1 halves of
        # src/x (needed a few passes in), then the late-used MLP weights.
        nc.sync.dma_start(out=wq_sb[:], in_=d_wq[:])
        nc.sync.dma_start(out=bq_sb[:], in_=d_bq[:])
        for q4 in range(4):
            nc.sync.dma_start(out=x_sb[:, 0, q4 * 512:(q4 + 1) * 512],
                              in_=d_x[:, 0, q4 * 512:(q4 + 1) * 512])
        nc.sync.dma_start(out=wk_sb[:], in_=d_wk[:])
        nc.sync.dma_start(out=bk_sb[:], in_=d_bk[:])
        for q4 in range(4):
            nc.sync.dma_start(out=src_sb[:, 0, q4 * 512:(q4 + 1) * 512],
                              in_=d_src[:, 0, q4 * 512:(q4 + 1) * 512])
        nc.sync.dma_start(out=wv_sb[:], in_=d_wv[:])
        nc.sync.dma_start(out=mask_sb[:, 0, :], in_=d_mask[:, 0, :])
        for q4 in range(2):
            nc.sync.dma_start(out=src_sb[:, 1, q4 * 512:(q4 + 1) * 512],
                              in_=d_src[:, 1, q4 * 512:(q4 + 1) * 512])
        nc.sync.dma_start(out=mask_sb[:, 1, :], in_=d_mask[:, 1, :])
        for q4 in range(2, 4):
            nc.sync.dma_start(out=src_sb[:, 1, q4 * 512:(q4 + 1) * 512],
                              in_=d_src[:, 1, q4 * 512:(q4 + 1) * 512])
        nc.sync.dma_start(out=mask_sb[:, 2, :], in_=d_mask[:, 2, :])
        for q4 in range(2):
            nc.sync.dma_start(out=x_sb[:, 1, q4 * 512:(q4 + 1) * 512],
                              in_=d_x[:, 1, q4 * 512:(q4 + 1) * 512])
        nc.sync.dma_start(out=mask_sb[:, 3, :], in_=d_mask[:, 3, :])
        for q4 in range(2, 4):
            nc.sync.dma_start(out=x_sb[:, 1, q4 * 512:(q4 + 1) * 512],
                              in_=d_x[:, 1, q4 * 512:(q4 + 1) * 512])
        for mc in range(4, 16):
            nc.sync.dma_start(out=mask_sb[:, mc, :], in_=d_mask[:, mc, :])
        for w_sb, d_w in ((wm_sb, d_wm), (bm_sb, d_bm), (w1_sb, d_w1),
                          (w2_sb, d_w2)):
            nc.sync.dma_start(out=w_sb[:], in_=d_w[:])

        nc.vector.memset(eps_sb[:], EPS)
        nc.vector.memset(vt_sb[:, :, :, DH:DH + 1], 1.0)
        # dummy exp: hoists the exp ACT table load off the window start
        nc.scalar.activation(scr_sb[:], eps_sb[:], AF.Exp)

        def bias_bcast(b_sb, oc, ncols):
            bb = b_sb[:, oc:oc + 1]
            return bass.AP(tensor=bb.tensor, offset=bb.offset,
                           ap=[list(bb.ap[0]), [0, ncols]])

        with tc.tile_pool(name="psA", bufs=2, space="PSUM") as psA, \
             tc.tile_pool(name="psB", bufs=3, space="PSUM") as psB, \
             tc.tile_pool(name="psC", bufs=1, space="PSUM") as psC:
            # ---- projections ----
            # chunk-0 bias-adds on DVE (idle pre-window); chunk-1 groups run
            # inside passes 0-1 with Scalar bias (DVE is saturated there)
            def proj_grp(w_sb, b_sb, rhs_sb, dst, oc, q4, dve_bias):
                pp = psB.tile([128, 512], F32, tag="psB")
                n0 = q4 * 512
                for kc in range(2):
                    nc.tensor.matmul(
                        pp[:],
                        lhsT=w_sb[:, kc, oc * 128:(oc + 1) * 128],
                        rhs=rhs_sb[:, kc, n0:n0 + 512],
                        start=(kc == 0), stop=(kc == 1))
                if dve_bias:
                    nc.vector.tensor_tensor(
                        dst[:, oc, n0:n0 + 512], pp[:],
                        bias_bcast(b_sb, oc, 512), op=ALU.add)
                else:
                    nc.scalar.activation(
                        dst[:, oc, n0:n0 + 512], pp[:],
                        AF.Identity, bias=b_sb[:, oc:oc + 1])

            def make_vt(mc):
                pv = psB.tile([128, 256], F32, tag="psB")
                for kc in range(2):
                    nc.tensor.matmul(
                        pv[:],
                        lhsT=src_sb[:, kc, mc * 128:(mc + 1) * 128],
                        rhs=wv_sb[:, kc, :],
                        start=(kc == 0), stop=(kc == 1))
                nc.scalar.activation(
                    vt_sb[:, mc, :, 0:DH],
                    pv[:].rearrange("p (h d) -> p h d", h=H), AF.Copy)

            for q4 in range(4):
                proj_grp(wq_sb, bq_sb, x_sb, q_sb, 0, q4, True)
            for q4 in range(4):
                proj_grp(wk_sb, bk_sb, src_sb, k_sb, 0, q4, True)
            for mc in range(16):
                make_vt(mc)

            # ---- merge / MLP1 half-0 sub-groups (512 cols, 1-bank psC) ----
            def merge_sub(oc, nq):
                mp = psC.tile([128, 512], F32, tag="psC")
                n0 = nq * 512
                for kc in range(2):
                    nc.tensor.matmul(
                        mp[:],
                        lhsT=wm_sb[:, kc, oc * 128:(oc + 1) * 128],
                        rhs=attn_sb[:, kc, n0:n0 + 512],
                        start=(kc == 0), stop=(kc == 1))
                nc.scalar.activation(
                    msg_sb[:, oc, n0:n0 + 512],
                    mp[:], AF.Identity, bias=bm_sb[:, oc:oc + 1])

            def y1_sub(oc, nq):
                yp = psC.tile([128, 512], F32, tag="psC")
                n0 = nq * 512
                for kc in range(4):
                    rhs_sb2 = x_sb if kc < 2 else msg_sb
                    nc.tensor.matmul(
                        yp[:],
                        lhsT=w1_sb[:, kc, oc * 128:(oc + 1) * 128],
                        rhs=rhs_sb2[:, kc % 2, n0:n0 + 512],
                        start=(kc == 0), stop=(kc == 3))
                nc.scalar.activation(
                    y1_sb[:, oc, n0:n0 + 512], yp[:], AF.Copy)

            fillers = {
                (0, 4): lambda: proj_grp(wq_sb, bq_sb, x_sb, q_sb, 1, 0, 0),
                (0, 8): lambda: proj_grp(wq_sb, bq_sb, x_sb, q_sb, 1, 1, 0),
                (0, 12): lambda: proj_grp(wq_sb, bq_sb, x_sb, q_sb, 1, 2, 0),
                (1, 4): lambda: proj_grp(wq_sb, bq_sb, x_sb, q_sb, 1, 3, 0),
                (1, 2): lambda: proj_grp(wk_sb, bk_sb, src_sb, k_sb, 1, 0, 0),
                (1, 6): lambda: proj_grp(wk_sb, bk_sb, src_sb, k_sb, 1, 1, 0),
                (1, 10): lambda: proj_grp(wk_sb, bk_sb, src_sb, k_sb, 1, 2,
                                          0),
                (1, 14): lambda: proj_grp(wk_sb, bk_sb, src_sb, k_sb, 1, 3,
                                          0),
                (5, 3): lambda: merge_sub(0, 0),
                (5, 7): lambda: merge_sub(0, 1),
                (5, 11): lambda: merge_sub(1, 0),
                (5, 15): lambda: merge_sub(1, 1),
                (6, 1): lambda: y1_sub(0, 0),
                (6, 5): lambda: y1_sub(0, 1),
                (6, 9): lambda: y1_sub(1, 0),
                (6, 13): lambda: y1_sub(1, 1),
                (7, 1): lambda: y1_sub(2, 0),
                (7, 5): lambda: y1_sub(2, 1),
                (7, 9): lambda: y1_sub(3, 0),
                (7, 13): lambda: y1_sub(3, 1),
            }

            # ---- attention ----
            passes = [(0, 0), (0, 1), (1, 0), (1, 1),
                      (0, 2), (0, 3), (1, 2), (1, 3)]
            pending = []            # (pt2, ap_e, ap_o, hc, mc_even)
            finish_q = []           # deferred reciprocal/normalize closures

            def flush_attn():
                pt2, ap_e, ap_o, hc, mce = pending.pop(0)
                for j in range(2):
                    mc = mce + j
                    nc.tensor.matmul(
                        ap_e[:], lhsT=vt_sb[:, mc, 2 * hc, :],
                        rhs=pt2[:, j * 1024:j * 1024 + 512],
                        start=(mc == 0), stop=(mc == 15))
                    nc.tensor.matmul(
                        ap_o[:], lhsT=vt_sb[:, mc, 2 * hc + 1, :],
                        rhs=pt2[:, j * 1024 + 512:(j + 1) * 1024],
                        start=(mc == 0), stop=(mc == 15))

            def drain_pass(ap_e, ap_o, hc, nq4, pi):
                # immediate: PSUM -> SBUF staging + exp-sum row to DRAM +
                # the [1,512]->[128,4] reshape DMA back in. Frees ap banks.
                n0 = nq4 * 512
                items = []
                for side, ap_t in ((0, ap_e), (1, ap_o)):
                    ri = pi * 2 + side
                    stg = stgp.tile([65, 512], F32, tag="stg")
                    nc.scalar.activation(stg[:], ap_t[:], AF.Copy)
                    nc.sync.dma_start(out=d_sums[ri:ri + 1, :],
                                      in_=stg[64:65, :])
                    rtmp = recp.tile([128, 4], F32, tag="rtmp")
                    nc.sync.dma_start(
                        out=rtmp[:],
                        in_=d_sums[ri:ri + 1, :].rearrange(
                            "a (p c) -> (a p) c", p=128))
                    items.append((side, stg, rtmp, ri))

                def finish():
                    for side, stg, rtmp, ri in items:
                        hp = side * 64
                        rcp = recp.tile([128, 4], F32, tag="rcp")
                        nc.vector.reciprocal(rcp[:], rtmp[:])
                        nc.sync.dma_start(
                            out=d_rscr[ri:ri + 1, :].rearrange(
                                "a (p c) -> (a p) c", p=128),
                            in_=rcp[:])
                        rsc = d_rscr.ap()
                        bcast = bass.AP(tensor=rsc.tensor, offset=ri * 512,
                                        ap=[[0, 64], [1, 512]])
                        rb = rbb.tile([64, 512], F32, tag="rb")
                        nc.sync.dma_start(out=rb[:], in_=bcast)
                        nc.gpsimd.tensor_tensor(
                            attn_sb[hp:hp + 64, hc, n0:n0 + 512],
                            stg[0:64, :], rb[:], op=ALU.mult)
                return finish

            for pi, (hc, nq4) in enumerate(passes):
                n0 = nq4 * 512
                ap_e = psB.tile([65, 512], F32, tag="psB")
                ap_o = psB.tile([65, 512], F32, tag="psB")
                pt = None
                for mc in range(16):
                    sp = psA.tile([128, 1024], F32, tag="psA")
                    nc.tensor.matmul(
                        sp[:, 0:512],
                        lhsT=k_sb[0:64, hc, mc * 128:(mc + 1) * 128],
                        rhs=q_sb[0:64, hc, n0:n0 + 512],
                        tile_position=(0, 0))
                    nc.tensor.matmul(
                        sp[:, 512:1024],
                        lhsT=k_sb[64:128, hc, mc * 128:(mc + 1) * 128],
                        rhs=q_sb[64:128, hc, n0:n0 + 512],
                        tile_position=(64, 0))
                    while len(pending) >= 2:
                        flush_attn()
                    if mc == 4 and finish_q:
                        finish_q.pop(0)()
                    if (pi, mc) in fillers:
                        fillers[(pi, mc)]()
                    if mc % 2 == 0:
                        pt = probp.tile([128, 2048], BF, tag="pt")
                    off = (mc % 2) * 1024
                    mrow = mask_sb[:, mc, n0:n0 + 512]
                    mb = bass.AP(tensor=mrow.tensor, offset=mrow.offset,
                                 ap=[list(mrow.ap[0]), [0, 2], [1, 512]])
                    nc.vector.tensor_tensor(
                        pt[:, off:off + 1024].rearrange(
                            "p (t n) -> p t n", t=2),
                        sp[:].rearrange("p (t n) -> p t n", t=2),
                        mb, op=ALU.mult)
                    if mc % 2 == 1:
                        pt2 = probp.tile([128, 2048], BF, tag="pt2")
                        nc.scalar.activation(pt2[:], pt[:], AF.Exp)
                        pending.append((pt2, ap_e, ap_o, hc, mc - 1))
                while pending:
                    flush_attn()
                finish_q.append(drain_pass(ap_e, ap_o, hc, nq4, pi))
                if pi == 7:
                    # dummy sqrt: loads the sqrt ACT table set (which also
                    # holds relu/copy/identity) behind the pass-7 drain
                    nc.scalar.activation(scr_sb[:], eps_sb[:], AF.Sqrt)
            while finish_q:
                finish_q.pop(0)()

            # ---- tail: merge h1, MLP1 h1, InstanceNorm, ReLU, MLP2 ----
            def merge_full(oc):
                mp = psA.tile([128, 1024], F32, tag="psA")
                for nq in range(2):
                    n0 = 1024 + nq * 512
                    for kc in range(2):
                        nc.tensor.matmul(
                            mp[:, nq * 512:(nq + 1) * 512],
                            lhsT=wm_sb[:, kc, oc * 128:(oc + 1) * 128],
                            rhs=attn_sb[:, kc, n0:n0 + 512],
                            start=(kc == 0), stop=(kc == 1))
                nc.scalar.activation(
                    msg_sb[:, oc, 1024:2048],
                    mp[:], AF.Identity, bias=bm_sb[:, oc:oc + 1])

            def y1_full(oc):
                yp = psA.tile([128, 1024], F32, tag="psA")
                for nq in range(2):
                    n0 = 1024 + nq * 512
                    for kc in range(4):
                        rhs_sb2 = x_sb if kc < 2 else msg_sb
                        nc.tensor.matmul(
                            yp[:, nq * 512:(nq + 1) * 512],
                            lhsT=w1_sb[:, kc, oc * 128:(oc + 1) * 128],
                            rhs=rhs_sb2[:, kc % 2, n0:n0 + 512],
                            start=(kc == 0), stop=(kc == 3))
                nc.scalar.activation(y1_sb[:, oc, 1024:2048], yp[:], AF.Copy)

            def y1_norm(oc):
                st = statp.tile([128, 4, 6], F32, tag="st")
                for half in range(2):
                    for nq in range(2):
                        n0 = half * 1024 + nq * 512
                        nc.vector.bn_stats(st[:, half * 2 + nq, :],
                                           y1_sb[:, oc, n0:n0 + 512])
                mv = statp.tile([128, 2], F32, tag="mv")
                nc.vector.bn_aggr(mv[:], st[:])
                sq = statp.tile([128, 1], F32, tag="sq")
                nc.scalar.activation(sq[:], mv[:, 1:2], AF.Sqrt,
                                     bias=eps_sb[:])
                rs = statp.tile([128, 1], F32, tag="rs")
                nc.vector.reciprocal(rs[:], sq[:])
                nb = statp.tile([128, 1], F32, tag="nb")
                nc.vector.scalar_tensor_tensor(nb[:], mv[:, 0:1], -1.0, rs[:],
                                               op0=ALU.mult, op1=ALU.mult)
                for half in range(2):
                    nc.scalar.activation(
                        y1n_sb[:, oc, half * 1024:(half + 1) * 1024],
                        y1_sb[:, oc, half * 1024:(half + 1) * 1024],
                        AF.Relu, bias=nb[:], scale=rs[:])

            merge_full(0)
            merge_full(1)
            y1_full(0)
            y1_full(1)
            y1_norm(0)
            y1_full(2)
            y1_norm(1)
            y1_full(3)
            y1_norm(2)
            y1_norm(3)

            for oc in range(2):
                for half in range(2):
                    op_t = psA.tile([128, 1024], F32, tag="psA")
                    for nq in range(2):
                        n0 = half * 1024 + nq * 512
                        for kc in range(4):
                            nc.tensor.matmul(
                                op_t[:, nq * 512:(nq + 1) * 512],
                                lhsT=w2_sb[:, kc, oc * 128:(oc + 1) * 128],
                                rhs=y1n_sb[:, kc, n0:n0 + 512],
                                start=(kc == 0), stop=(kc == 3))
                    o_sb = outp.tile([128, 1024], F32, tag="outsb")
                    nc.vector.tensor_copy(o_sb[:], op_t[:])
                    n0 = half * 1024
                    nc.sync.dma_start(out=d_out[:, oc, n0:n0 + 512],
                                      in_=o_sb[:, 0:512])
                    nc.sync.dma_start(out=d_out[:, oc, n0 + 512:n0 + 1024],
                                      in_=o_sb[:, 512:1024])

    nc.compile()
    return nc


def _chunk(a, p=128):
    # [C, ...] -> [128, C//128, ...] with partition-major layout
    c = a.shape[0]
    return np.ascontiguousarray(
        a.reshape(c // p, p, *a.shape[1:]).swapaxes(0, 1))


def _prep_inputs(x, source, mask, Wq, bq, Wk, bk, Wv, bv, Wm, bm, W1, b1,
                 W2, b2):
    # blocked-head channel permutation: new[h*64+d] = old[d*4+h]
    perm = (np.arange(DH)[None, :] * H + np.arange(H)[:, None]).reshape(-1)
    scale = 1.0 / np.sqrt(np.float32(DH))

    wq_t = _chunk((Wq[perm, :] * scale).T.astype(NPBF))
    wk_t = _chunk(Wk[perm, :].T.astype(NPBF))
    wv_t = _chunk(Wv[perm, :].T.astype(NPBF))
    wm_t = _chunk(Wm[:, perm].T.astype(NPBF))
    w1_t = _chunk(W1.T.astype(NPBF))
    w2_t = _chunk(W2.T.astype(NPBF))
    bq_t = _chunk((bq[perm] * scale).astype(np.float32))
    bk_t = _chunk(bk[perm].astype(np.float32))
    bm_t = _chunk((Wm @ bv + bm).astype(np.float32))

    shared = {"wqT": wq_t, "wkT": wk_t, "wvT": wv_t, "wmT": wm_t,
              "w1T": w1_t, "w2T": w2_t, "bq": bq_t, "bk": bk_t, "bmE": bm_t}

    in_maps = []
    for b in range(B):
        m = dict(shared)
        m["x"] = _chunk(np.asarray(x[b]).astype(NPBF))
        m["src"] = _chunk(np.asarray(source[b]).astype(NPF8))
        m["maskT"] = _chunk(np.ascontiguousarray(
            np.asarray(mask[b]).T).astype(NPF8))
        in_maps.append(m)
    return in_maps


def run(inputs, trace=False):
    if "nc" not in _CACHE:
        _CACHE["nc"] = _build()
    nc = _CACHE["nc"]
    in_maps = _prep_inputs(**inputs)
    res = run_bass_kernel_spmd(nc, in_maps, list(range(NCORES)), trace=trace)
    out = np.empty((B, D, N), np.float32)
    for b in range(B):
        o = res.results[b]["out"]  # [128, 2, N]
        out[b] = o.swapaxes(0, 1).reshape(D, N)
    return out, res


def kernel(**inputs):
    out, _ = run(inputs, trace=False)
    return out


# revision 9
# speedup vs baseline: 1.1716x; 1.0546x over previous
"""Trainium2 Bass kernel for AdaAttentionalPropagation (masked multi-head
cross-attention + merge conv + MLP with InstanceNorm/ReLU).

Full inputs in, full output out. Internally: data-parallel over batch B=8
across 8 NeuronCores (one batch element per core, no collectives).

Math notes (host-side folds, all exact):
  - head channels are re-permuted to blocked layout (h*64+d) by permuting
    Wq/Wk/Wv rows and Wm columns
  - 1/sqrt(dh) is folded into Wq and bq
  - bv folds into an effective merge bias bmE = Wm@bv + bm (softmax rows sum
    to 1, so v's bias contributes Wm@bv to the message)
  - b1 is dropped: a per-channel constant cancels in InstanceNorm(affine=False)
  - softmax is computed without max-subtraction (scores are O(1) here)
  - softmax denominator comes free from a ones-column appended to v^T in the
    attention matmul (row 64 of the PSUM accumulator)
  - mask and source are carried in fp8e4m3 (validated: ~4e-4 rel err end to
    end); shrinks the dominant input DMA so the pipeline start isn't gated

Schedule notes:
  - window = 128 iterations of {scores MM pair (row-tiled, concurrent),
    DVE mask-mult from PSUM (~1142ns, the pipeline governor), Scalar exp on
    [128,2048] double-tiles, trailing attention MMs}
  - input DMA is split along N so projections start on the first slices
  - prologue computes q/k chunk 0 (DVE bias-adds) + vT; q/k output-chunk 1
    is projected inside passes 0-1 from PE slack (Scalar bias)
  - per-pass accumulator drains are immediate (3-slot psB rotation never
    blocks the next pass); reciprocal/normalize finish is deferred ~4
    iterations into the next pass (DRAM round trip for the [1,512]->[128,4]
    reshape; multiply on the otherwise-idle GpSimd)
  - merge conv h0 + MLP1 (oc 0-1, h0) + merge q2 run inside passes 5-7
  - pass 7 drains via a sums-broadcast DMA + reciprocal_approx_fast + DVE
    normalize (one DRAM round trip instead of two); the round-trip shadow
    is filled with MLP1 (oc 2-3 h0, q2) matmuls and InstanceNorm stats
  - ReLU is split: h1 on Scalar (fused affine), h0 on DVE (tensor_scalar +
    max); MLP2 accumulates each kc as soon as that channel's ReLU lands
"""

import sys

for _p in ("/opt/trn_rl_repo", "/root/.axon_site/_ro/trn_rl_repo"):
    if _p not in sys.path:
        sys.path.append(_p)

import numpy as np
import ml_dtypes
from contextlib import ExitStack

import concourse.bass as bass
import concourse.tile as tile
from concourse import bacc, mybir
from concourse.bass_utils import run_bass_kernel_spmd

B, D, N, NKV, H = 8, 256, 2048, 2048, 4
DH = D // H
EPS = 1e-5
NCORES = 8

BF = mybir.dt.bfloat16
F32 = mybir.dt.float32
F8 = mybir.dt.float8e4
AF = mybir.ActivationFunctionType
ALU = mybir.AluOpType
NPBF = ml_dtypes.bfloat16
NPF8 = ml_dtypes.float8_e4m3

_CACHE = {}


def _build():
    nc = bacc.Bacc("TRN2", target_bir_lowering=False, debug=False,
                   num_devices=NCORES)

    d_x = nc.dram_tensor("x", [128, 2, N], BF, kind="ExternalInput")
    d_src = nc.dram_tensor("src", [128, 2, N], F8, kind="ExternalInput")
    d_mask = nc.dram_tensor("maskT", [128, 16, N], F8, kind="ExternalInput")
    d_wq = nc.dram_tensor("wqT", [128, 2, 256], BF, kind="ExternalInput")
    d_wk = nc.dram_tensor("wkT", [128, 2, 256], BF, kind="ExternalInput")
    d_wv = nc.dram_tensor("wvT", [128, 2, 256], BF, kind="ExternalInput")
    d_wm = nc.dram_tensor("wmT", [128, 2, 256], BF, kind="ExternalInput")
    d_w1 = nc.dram_tensor("w1T", [128, 4, 512], BF, kind="ExternalInput")
    d_w2 = nc.dram_tensor("w2T", [128, 4, 256], BF, kind="ExternalInput")
    d_bq = nc.dram_tensor("bq", [128, 2], F32, kind="ExternalInput")
    d_bk = nc.dram_tensor("bk", [128, 2], F32, kind="ExternalInput")
    d_bm = nc.dram_tensor("bmE", [128, 2], F32, kind="ExternalInput")
    d_out = nc.dram_tensor("out", [128, 2, N], F32, kind="ExternalOutput")
    d_rscr = nc.dram_tensor("rscratch", [16, 512], F32)
    d_sums = nc.dram_tensor("sscratch", [16, 512], F32)

    with tile.TileContext(nc) as tc, ExitStack() as ctx:
        consts = ctx.enter_context(tc.tile_pool(name="consts", bufs=1))
        probp = ctx.enter_context(tc.tile_pool(name="probp", bufs=6))
        recp = ctx.enter_context(tc.tile_pool(name="recp", bufs=2))
        rbb = ctx.enter_context(tc.tile_pool(name="rbb", bufs=3))
        stgp = ctx.enter_context(tc.tile_pool(name="stgp", bufs=4))
        statp = ctx.enter_context(tc.tile_pool(name="statp", bufs=10))
        outp = ctx.enter_context(tc.tile_pool(name="outp", bufs=2))

        wq_sb = consts.tile([128, 2, 256], BF)
        wk_sb = consts.tile([128, 2, 256], BF)
        wv_sb = consts.tile([128, 2, 256], BF)
        wm_sb = consts.tile([128, 2, 256], BF)
        w1_sb = consts.tile([128, 4, 512], BF)
        w2_sb = consts.tile([128, 4, 256], BF)
        bq_sb = consts.tile([128, 2], F32)
        bk_sb = consts.tile([128, 2], F32)
        bm_sb = consts.tile([128, 2], F32)
        x_sb = consts.tile([128, 2, N], BF)
        src_sb = consts.tile([128, 2, N], F8)
        mask_sb = consts.tile([128, 16, N], F8)
        q_sb = consts.tile([128, 2, N], BF)
        k_sb = consts.tile([128, 2, N], BF)
        vt_sb = consts.tile([128, 16, H, DH + 1], BF)
        attn_sb = consts.tile([128, 2, N], BF)
        msg_sb = consts.tile([128, 2, N], BF)
        y1_sb = consts.tile([128, 4, N], BF)
        y1n_sb = consts.tile([128, 4, N], BF)
        eps_sb = consts.tile([128, 1], F32)
        scr_sb = consts.tile([128, 1], F32)

        # ---- input DMA. Split along N so the first projections can start
        # after the first slices; mask chunks stream behind, ahead of their
        # pass-0 consumption; late-used MLP weights go last.
        def dx(kc, s):
            nc.sync.dma_start(out=x_sb[:, kc, s * 512:(s + 1) * 512],
                              in_=d_x[:, kc, s * 512:(s + 1) * 512])

        def ds(kc, s):
            nc.sync.dma_start(out=src_sb[:, kc, s * 512:(s + 1) * 512],
                              in_=d_src[:, kc, s * 512:(s + 1) * 512])

        def dm(mc):
            nc.sync.dma_start(out=mask_sb[:, mc, :], in_=d_mask[:, mc, :])

        nc.sync.dma_start(out=wq_sb[:], in_=d_wq[:])
        nc.sync.dma_start(out=bq_sb[:], in_=d_bq[:])
        dx(0, 0)
        dx(1, 0)
        nc.sync.dma_start(out=wk_sb[:], in_=d_wk[:])
        nc.sync.dma_start(out=bk_sb[:], in_=d_bk[:])
        ds(0, 0)
        ds(1, 0)
        ds(0, 1)
        ds(1, 1)
        nc.sync.dma_start(out=wv_sb[:], in_=d_wv[:])
        ds(0, 2)
        ds(1, 2)
        ds(0, 3)
        ds(1, 3)
        dm(0)
        dm(1)
        dx(0, 1)
        dx(1, 1)
        dm(2)
        dx(0, 2)
        dx(1, 2)
        dm(3)
        dx(0, 3)
        dx(1, 3)
        for mc in range(4, 16):
            dm(mc)
        for w_sb, d_w in ((wm_sb, d_wm), (bm_sb, d_bm), (w1_sb, d_w1),
                          (w2_sb, d_w2)):
            nc.sync.dma_start(out=w_sb[:], in_=d_w[:])

        nc.vector.memset(eps_sb[:], EPS)
        nc.vector.memset(vt_sb[:, :, :, DH:DH + 1], 1.0)
        # dummy exp: hoists the exp ACT table load off the window start
        nc.scalar.activation(scr_sb[:], eps_sb[:], AF.Exp)

        def bias_bcast(b_sb, oc, ncols):
            bb = b_sb[:, oc:oc + 1]
            return bass.AP(tensor=bb.tensor, offset=bb.offset,
                           ap=[list(bb.ap[0]), [0, ncols]])

        with tc.tile_pool(name="psA", bufs=2, space="PSUM") as psA, \
             tc.tile_pool(name="psB", bufs=3, space="PSUM") as psB, \
             tc.tile_pool(name="psC", bufs=1, space="PSUM") as psC:
            # ---- projections ----
            def proj_grp(w_sb, b_sb, rhs_sb, dst, oc, q4, dve_bias):
                pp = psB.tile([128, 512], F32, tag="psB")
                n0 = q4 * 512
                for kc in range(2):
                    nc.tensor.matmul(
                        pp[:],
                        lhsT=w_sb[:, kc, oc * 128:(oc + 1) * 128],
                        rhs=rhs_sb[:, kc, n0:n0 + 512],
                        start=(kc == 0), stop=(kc == 1))
                if dve_bias:
                    nc.vector.tensor_tensor(
                        dst[:, oc, n0:n0 + 512], pp[:],
                        bias_bcast(b_sb, oc, 512), op=ALU.add)
                else:
                    nc.scalar.activation(
                        dst[:, oc, n0:n0 + 512], pp[:],
                        AF.Identity, bias=b_sb[:, oc:oc + 1])

            def make_vt(mc):
                pv = psB.tile([128, 256], F32, tag="psB")
                for kc in range(2):
                    nc.tensor.matmul(
                        pv[:],
                        lhsT=src_sb[:, kc, mc * 128:(mc + 1) * 128],
                        rhs=wv_sb[:, kc, :],
                        start=(kc == 0), stop=(kc == 1))
                nc.scalar.activation(
                    vt_sb[:, mc, :, 0:DH],
                    pv[:].rearrange("p (h d) -> p h d", h=H), AF.Copy)

            proj_grp(wq_sb, bq_sb, x_sb, q_sb, 0, 0, True)
            for q4 in range(2):
                proj_grp(wk_sb, bk_sb, src_sb, k_sb, 0, q4, True)
            for mc in range(8):
                make_vt(mc)
            for q4 in range(2, 4):
                proj_grp(wk_sb, bk_sb, src_sb, k_sb, 0, q4, True)
            for mc in range(8, 16):
                make_vt(mc)
            for q4 in range(1, 4):
                proj_grp(wq_sb, bq_sb, x_sb, q_sb, 0, q4, True)

            # ---- merge / MLP1 fillers and tail groups ----
            def merge_sub(oc, nq):
                mp = psC.tile([128, 512], F32, tag="psC")
                n0 = nq * 512
                for kc in range(2):
                    nc.tensor.matmul(
                        mp[:],
                        lhsT=wm_sb[:, kc, oc * 128:(oc + 1) * 128],
                        rhs=attn_sb[:, kc, n0:n0 + 512],
                        start=(kc == 0), stop=(kc == 1))
                nc.scalar.activation(
                    msg_sb[:, oc, n0:n0 + 512],
                    mp[:], AF.Identity, bias=bm_sb[:, oc:oc + 1])

            def y1_mms(yp, oc, n0, w):
                for kc in range(4):
                    rhs_sb2 = x_sb if kc < 2 else msg_sb
                    nc.tensor.matmul(
                        yp[:, 0:w] if w == 512 else yp[:],
                        lhsT=w1_sb[:, kc, oc * 128:(oc + 1) * 128],
                        rhs=rhs_sb2[:, kc % 2, n0:n0 + w],
                        start=(kc == 0), stop=(kc == 3))

            def y1_sub(oc, nq, pool, tag):
                yp = pool.tile([128, 512], F32, tag=tag)
                n0 = nq * 512
                y1_mms(yp, oc, n0, 512)
                nc.scalar.activation(
                    y1_sb[:, oc, n0:n0 + 512], yp[:], AF.Copy)

            def y1_half(oc, half, pool):
                yp = pool.tile([128, 1024], F32, tag="psA")
                n0 = half * 1024
                for nq in range(2):
                    for kc in range(4):
                        rhs_sb2 = x_sb if kc < 2 else msg_sb
                        nc.tensor.matmul(
                            yp[:, nq * 512:(nq + 1) * 512],
                            lhsT=w1_sb[:, kc, oc * 128:(oc + 1) * 128],
                            rhs=rhs_sb2[:, kc % 2,
                                        n0 + nq * 512:n0 + (nq + 1) * 512],
                            start=(kc == 0), stop=(kc == 3))
                nc.scalar.activation(
                    y1_sb[:, oc, n0:n0 + 1024], yp[:], AF.Copy)

            fillers = {
                (0, 4): lambda: proj_grp(wq_sb, bq_sb, x_sb, q_sb, 1, 0, 0),
                (0, 8): lambda: proj_grp(wq_sb, bq_sb, x_sb, q_sb, 1, 1, 0),
                (0, 12): lambda: proj_grp(wq_sb, bq_sb, x_sb, q_sb, 1, 2, 0),
                (1, 4): lambda: proj_grp(wq_sb, bq_sb, x_sb, q_sb, 1, 3, 0),
                (1, 2): lambda: proj_grp(wk_sb, bk_sb, src_sb, k_sb, 1, 0, 0),
                (1, 6): lambda: proj_grp(wk_sb, bk_sb, src_sb, k_sb, 1, 1, 0),
                (1, 10): lambda: proj_grp(wk_sb, bk_sb, src_sb, k_sb, 1, 2,
                                          0),
                (1, 14): lambda: proj_grp(wk_sb, bk_sb, src_sb, k_sb, 1, 3,
                                          0),
                (5, 3): lambda: merge_sub(0, 0),
                (5, 7): lambda: merge_sub(0, 1),
                (5, 11): lambda: merge_sub(1, 0),
                (5, 15): lambda: merge_sub(1, 1),
                (6, 1): lambda: y1_sub(0, 0, psC, "psC"),
                (6, 5): lambda: y1_sub(0, 1, psC, "psC"),
                (6, 9): lambda: y1_sub(1, 0, psC, "psC"),
                (6, 13): lambda: y1_sub(1, 1, psC, "psC"),
                (7, 9): lambda: merge_sub(0, 2),
                (7, 11): lambda: merge_sub(1, 2),
            }

            # ---- attention ----
            passes = [(0, 0), (0, 1), (1, 0), (1, 1),
                      (0, 2), (0, 3), (1, 2), (1, 3)]
            pending = []            # (pt2, ap_e, ap_o, hc, mc_even)
            finish_q = []           # deferred reciprocal/normalize closures

            def flush_attn():
                pt2, ap_e, ap_o, hc, mce = pending.pop(0)
                for j in range(2):
                    mc = mce + j
                    nc.tensor.matmul(
                        ap_e[:], lhsT=vt_sb[:, mc, 2 * hc, :],
                        rhs=pt2[:, j * 1024:j * 1024 + 512],
                        start=(mc == 0), stop=(mc == 15))
                    nc.tensor.matmul(
                        ap_o[:], lhsT=vt_sb[:, mc, 2 * hc + 1, :],
                        rhs=pt2[:, j * 1024 + 512:(j + 1) * 1024],
                        start=(mc == 0), stop=(mc == 15))

            def drain_pass(ap_e, ap_o, hc, nq4, pi):
                # immediate: PSUM -> SBUF staging + exp-sum row to DRAM +
                # the [1,512]->[128,4] reshape DMA back in. Frees ap banks.
                n0 = nq4 * 512
                items = []
                for side, ap_t in ((0, ap_e), (1, ap_o)):
                    ri = pi * 2 + side
                    stg = stgp.tile([65, 512], F32, tag="stg")
                    nc.scalar.activation(stg[:], ap_t[:], AF.Copy)
                    nc.sync.dma_start(out=d_sums[ri:ri + 1, :],
                                      in_=stg[64:65, :])
                    rtmp = recp.tile([128, 4], F32, tag="rtmp")
                    nc.sync.dma_start(
                        out=rtmp[:],
                        in_=d_sums[ri:ri + 1, :].rearrange(
                            "a (p c) -> (a p) c", p=128))
                    items.append((side, stg, rtmp, ri))

                def finish():
                    for side, stg, rtmp, ri in items:
                        hp = side * 64
                        rcp = recp.tile([128, 4], F32, tag="rcp")
                        nc.vector.reciprocal(rcp[:], rtmp[:])
                        nc.sync.dma_start(
                            out=d_rscr[ri:ri + 1, :].rearrange(
                                "a (p c) -> (a p) c", p=128),
                            in_=rcp[:])
                        rsc = d_rscr.ap()
                        bcast = bass.AP(tensor=rsc.tensor, offset=ri * 512,
                                        ap=[[0, 64], [1, 512]])
                        rb = rbb.tile([64, 512], F32, tag="rb")
                        nc.sync.dma_start(out=rb[:], in_=bcast)
                        nc.gpsimd.tensor_tensor(
                            attn_sb[hp:hp + 64, hc, n0:n0 + 512],
                            stg[0:64, :], rb[:], op=ALU.mult)
                return finish

            def drain_fast(ap_e, ap_o, hc, nq4, pi):
                # tail variant: one DRAM round trip (broadcast the sums, not
                # the reciprocals) + reciprocal_approx_fast + DVE normalize
                n0 = nq4 * 512
                items = []
                for side, ap_t in ((0, ap_e), (1, ap_o)):
                    ri = pi * 2 + side
                    stg = stgp.tile([65, 512], F32, tag="stg")
                    nc.scalar.activation(stg[:], ap_t[:], AF.Copy)
                    nc.sync.dma_start(out=d_sums[ri:ri + 1, :],
                                      in_=stg[64:65, :])
                    items.append((side, stg, ri))
                reps = []
                for side, stg, ri in items:
                    rsc = d_sums.ap()
                    bcast = bass.AP(tensor=rsc.tensor, offset=ri * 512,
                                    ap=[[0, 64], [1, 512]])
                    srep = rbb.tile([64, 512], F32, tag="rb")
                    nc.sync.dma_start(out=srep[:], in_=bcast)
                    reps.append(srep)
                for (side, stg, ri), srep in zip(items, reps):
                    hp = side * 64
                    rinv = rbb.tile([64, 512], F32, tag="rb")
                    nc.vector.reciprocal_approx_fast(rinv[:], srep[:])
                    nc.vector.tensor_tensor(
                        attn_sb[hp:hp + 64, hc, n0:n0 + 512],
                        stg[0:64, :], rinv[:], op=ALU.mult)

            last_pt2 = None
            for pi, (hc, nq4) in enumerate(passes):
                n0 = nq4 * 512
                ap_e = psB.tile([65, 512], F32, tag="psB")
                ap_o = psB.tile([65, 512], F32, tag="psB")
                pt = None
                for mc in range(16):
                    sp = psA.tile([128, 1024], F32, tag="psA")
                    nc.tensor.matmul(
                        sp[:, 0:512],
                        lhsT=k_sb[0:64, hc, mc * 128:(mc + 1) * 128],
                        rhs=q_sb[0:64, hc, n0:n0 + 512],
                        tile_position=(0, 0))
                    nc.tensor.matmul(
                        sp[:, 512:1024],
                        lhsT=k_sb[64:128, hc, mc * 128:(mc + 1) * 128],
                        rhs=q_sb[64:128, hc, n0:n0 + 512],
                        tile_position=(64, 0))
                    while len(pending) >= 2:
                        flush_attn()
                    if mc == 4 and finish_q:
                        finish_q.pop(0)()
                    if (pi, mc) in fillers:
                        fillers[(pi, mc)]()
                    if mc % 2 == 0:
                        pt = probp.tile([128, 2048], BF, tag="pt")
                    off = (mc % 2) * 1024
                    mrow = mask_sb[:, mc, n0:n0 + 512]
                    mb = bass.AP(tensor=mrow.tensor, offset=mrow.offset,
                                 ap=[list(mrow.ap[0]), [0, 2], [1, 512]])
                    nc.vector.tensor_tensor(
                        pt[:, off:off + 1024].rearrange(
                            "p (t n) -> p t n", t=2),
                        sp[:].rearrange("p (t n) -> p t n", t=2),
                        mb, op=ALU.mult)
                    if mc % 2 == 1:
                        pt2 = probp.tile([128, 2048], BF, tag="pt")
                        nc.scalar.activation(pt2[:], pt[:], AF.Exp)
                        pending.append((pt2, ap_e, ap_o, hc, mc - 1))
                        last_pt2 = pt2
                while pending:
                    flush_attn()
                if pi < 7:
                    finish_q.append(drain_pass(ap_e, ap_o, hc, nq4, pi))
                else:
                    drain_fast(ap_e, ap_o, hc, nq4, pi)
                    # anchored on the last exp output so the scheduler can't
                    # hoist it: loads the sqrt ACT table set (which also has
                    # relu/copy/identity) while the drain round trip flies
                    nc.scalar.activation(scr_sb[:], last_pt2[:, 0:1],
                                         AF.Sqrt)
            while finish_q:
                finish_q.pop(0)()

            # ---- tail ----
            # round-trip shadow work: MLP1 h0 for oc 2-3, MLP1 q2, stats
            stats = {}

            def st_of(oc):
                if oc not in stats:
                    st_t = statp.tile([128, 4, 6], F32, tag="st")
                    stats[oc] = st_t
                return stats[oc]

            def q_stats(oc, q):
                nc.vector.bn_stats(st_of(oc)[:, q, :],
                                   y1_sb[:, oc, q * 512:(q + 1) * 512])

            y1_half(2, 0, psA)
            y1_half(3, 0, psA)
            for oc in range(4):
                y1_sub(oc, 2, psC, "psC")
            for oc in range(4):
                for q in range(2):
                    q_stats(oc, q)
            for oc in range(4):
                q_stats(oc, 2)
            # gated by the pass-7 normalize:
            merge_sub(0, 3)
            merge_sub(1, 3)
            for oc in range(4):
                y1_sub(oc, 3, psA, "psA")
                q_stats(oc, 3)

            # InstanceNorm scale/shift + split ReLU + MLP2
            rs_l, nb_l = [], []
            for oc in range(4):
                mv = statp.tile([128, 2], F32, tag="mv")
                nc.vector.bn_aggr(mv[:], st_of(oc)[:])
                sq = statp.tile([128, 1], F32, tag="sq")
                nc.scalar.activation(sq[:], mv[:, 1:2], AF.Sqrt,
                                     bias=eps_sb[:])
                rs = statp.tile([128, 1], F32, tag="rs")
                nc.vector.reciprocal(rs[:], sq[:])
                nb = statp.tile([128, 1], F32, tag="nb")
                nc.vector.scalar_tensor_tensor(nb[:], mv[:, 0:1], -1.0, rs[:],
                                               op0=ALU.mult, op1=ALU.mult)
                rs_l.append(rs)
                nb_l.append(nb)

            for oc in range(4):
                # h1 on Scalar (fused affine+relu); h0 on DVE
                nc.scalar.activation(
                    y1n_sb[:, oc, 1024:2048], y1_sb[:, oc, 1024:2048],
                    AF.Relu, bias=nb_l[oc][:], scale=rs_l[oc][:])
                tmp = outp.tile([128, 1024], BF, tag="outsb")
                nc.vector.tensor_scalar(
                    tmp[:], y1_sb[:, oc, 0:1024],
                    rs_l[oc][:, 0:1], nb_l[oc][:, 0:1],
                    op0=ALU.mult, op1=ALU.add)
                nc.vector.tensor_scalar_max(
                    y1n_sb[:, oc, 0:1024], tmp[:], 0.0)

            for oc in range(2):
                for half in range(2):
                    op_t = psA.tile([128, 1024], F32, tag="psA")
                    for kc in range(4):
                        for nq in range(2):
                            n0 = half * 1024 + nq * 512
                            nc.tensor.matmul(
                                op_t[:, nq * 512:(nq + 1) * 512],
                                lhsT=w2_sb[:, kc, oc * 128:(oc + 1) * 128],
                                rhs=y1n_sb[:, kc, n0:n0 + 512],
                                start=(kc == 0), stop=(kc == 3))
                    o_sb = outp.tile([128, 1024], F32, tag="outsb")
                    nc.vector.tensor_copy(o_sb[:], op_t[:])
                    n0 = half * 1024
                    nc.sync.dma_start(out=d_out[:, oc, n0:n0 + 512],
                                      in_=o_sb[:, 0:512])
                    nc.sync.dma_start(out=d_out[:, oc, n0 + 512:n0 + 1024],
                                      in_=o_sb[:, 512:1024])

    nc.compile()
    return nc


def _chunk(a, p=128):
    # [C, ...] -> [128, C//128, ...] with partition-major layout
    c = a.shape[0]
    return np.ascontiguousarray(
        a.reshape(c // p, p, *a.shape[1:]).swapaxes(0, 1))


def _prep_inputs(x, source, mask, Wq, bq, Wk, bk, Wv, bv, Wm, bm, W1, b1,
                 W2, b2):
    # blocked-head channel permutation: new[h*64+d] = old[d*4+h]
    perm = (np.arange(DH)[None, :] * H + np.arange(H)[:, None]).reshape(-1)
    scale = 1.0 / np.sqrt(np.float32(DH))

    wq_t = _chunk((Wq[perm, :] * scale).T.astype(NPBF))
    wk_t = _chunk(Wk[perm, :].T.astype(NPBF))
    wv_t = _chunk(Wv[perm, :].T.astype(NPBF))
    wm_t = _chunk(Wm[:, perm].T.astype(NPBF))
    w1_t = _chunk(W1.T.astype(NPBF))
    w2_t = _chunk(W2.T.astype(NPBF))
    bq_t = _chunk((bq[perm] * scale).astype(np.float32))
    bk_t = _chunk(bk[perm].astype(np.float32))
    bm_t = _chunk((Wm @ bv + bm).astype(np.float32))

    shared = {"wqT": wq_t, "wkT": wk_t, "wvT": wv_t, "wmT": wm_t,
              "w1T": w1_t, "w2T": w2_t, "bq": bq_t, "bk": bk_t, "bmE": bm_t}

    in_maps = []
    for b in range(B):
        m = dict(shared)
        m["x"] = _chunk(np.asarray(x[b]).astype(NPBF))
        m["src"] = _chunk(np.asarray(source[b]).astype(NPF8))
        m["maskT"] = _chunk(np.ascontiguousarray(
            np.asarray(mask[b]).T).astype(NPF8))
        in_maps.append(m)
    return in_maps


def run(inputs, trace=False):
    if "nc" not in _CACHE:
        _CACHE["nc"] = _build()
    nc = _CACHE["nc"]
    in_maps = _prep_inputs(**inputs)
    res = run_bass_kernel_spmd(nc, in_maps, list(range(NCORES)), trace=trace)
    out = np.empty((B, D, N), np.float32)
    for b in range(B):
        o = res.results[b]["out"]  # [128, 2, N]
        out[b] = o.swapaxes(0, 1).reshape(D, N)
    return out, res


def kernel(**inputs):
    out, _ = run(inputs, trace=False)
    return out
